# revision 1
# baseline (speedup 1.0000x reference)
"""Trainium2 Bass kernel for nn_Encoder_trace (GNN message passing + cross-attention).

Data-parallel over the batch axis B=64 across 8 NeuronCores (8 graphs/core).
Device layout: channels on SBUF partitions, tokens on the free dimension
(everything computed transposed; host un-transposes on gather).

All weight-combination products AND the chain-GCN token aggregation are
precomputed on the HOST; every matmul operand is bf16.  Per graph the device
runs only the streaming work, software-pipelined so the front matmuls of
graph g+1 and the out-projection of graph g-1 interleave with the attention
pairs of graph g (PE filler while ACT digests exp and DVE the normalize).

Math per graph (g), with xa = agg(x) done on host:
  x_timeT = W_comb @ xaT + bxtf            (W_comb  = W_gcn W_lin)
  qT      = W_qcomb @ xaT + bqc            (W_qcomb = Wq W_gcn W_lin)
  kT      = Wk @ WE + bk                   [host]
  vvh     = WE.T @ Wv.T, ones col interleaved per head  [host]
  per head: scoresT = kT_h.T @ qT_h ; exp (ACT, scale=1/8, no max-sub)
            op[0:65] = [vvh_h | 1].T @ exp (row 64 = softmax sums, free)
            rc = recip(op[64]) ; rcb = ones-matmul broadcast (tile_position)
            staged to SBUF (hw: one PSUM operand per DVE op)
            oT = op[0:64] * rcb
  x_outT  = W_out @ oT + boute             (boute = W_out bv + b_out)
"""

import numpy as np
from contextlib import ExitStack

import concourse.bass as bass
import concourse.mybir as mybir
import concourse.tile as tile
from concourse.bass import ts, ds

# problem dims (hardcoded per spec)
B, F, D, H, NH, DH, V = 64, 512, 256, 768, 12, 64, 256
NCORES = 8
G = B // NCORES       # graphs per core
KH = H // 128         # 6  (H in 128-partition tiles)
KD = D // 128         # 2  (D in 128-partition tiles)
NPAIR = NH // 2       # 6  head pairs

F32 = mybir.dt.float32
BF16 = mybir.dt.bfloat16
AF = mybir.ActivationFunctionType
ALU = mybir.AluOpType

WT = BF16
WT_NP = mybir.dt.np(WT)

RSQRT2 = float(2.0 ** -0.5)
C1 = RSQRT2 - 0.5  # chain-GCN col-1 colsum deviation (bias correction coeff)


def build_program(bias_fix=False):
    nc = bass.Bass()

    xt_d = nc.declare_dram_parameter("xt", [G, D, F], WT, isOutput=False)
    wqc_d = nc.declare_dram_parameter("w_qcomb", [D, H], WT, isOutput=False)
    wcb_d = nc.declare_dram_parameter("w_comb", [D, H], WT, isOutput=False)
    wot_d = nc.declare_dram_parameter("w_out_t", [H, H], WT, isOutput=False)
    kt_d = nc.declare_dram_parameter("k_t", [H, V], WT, isOutput=False)
    vva_d = nc.declare_dram_parameter("vvh_a", [D, NH * (DH + 1)], WT, isOutput=False)
    NB = 5 if bias_fix else 3
    bp_d = nc.declare_dram_parameter("b_pack", [128, KH * NB], F32, isOutput=False)
    ones_d = nc.declare_dram_parameter("ones_r", [1, DH], WT, isOutput=False)
    oxt_d = nc.declare_dram_parameter("out_xt", [G, H, F], BF16, isOutput=True)
    oxo_d = nc.declare_dram_parameter("out_xo", [G, H, F], BF16, isOutput=True)

    with ExitStack() as ctx:
        tc = ctx.enter_context(tile.TileContext(nc))
        wp = ctx.enter_context(tc.tile_pool(name="wp", bufs=1))
        pp = ctx.enter_context(tc.tile_pool(name="pp", bufs=1, space="PSUM"))
        dp = ctx.enter_context(tc.tile_pool(name="dp", bufs=1))

        def ptile(shape, tag, bufs):
            return pp.tile(shape, F32, name=tag, tag=tag, bufs=bufs)

        def wtile(shape, dt, tag):
            return wp.tile(shape, dt, name=tag, tag=tag)


        # ---------------- persistent weights (DMA in consumer order) -------
        wqc = [wtile([128, H], WT, f"wqc{k}") for k in range(KD)]
        wcb = [wtile([128, H], WT, f"wcb{k}") for k in range(KD)]
        kt = [wtile([128, V], WT, f"kt{m}") for m in range(KH)]
        vva = [wtile([128, NH * (DH + 1)], WT, f"vva{m}") for m in range(KD)]
        wout = [wtile([128, H], WT, f"wout{k}") for k in range(KH)]
        onesb = wtile([1, DH], WT, "onesb")
        bpack = wtile([128, KH * NB], F32, "bpack")
        bqc = [bpack[:, m * NB : m * NB + 1] for m in range(KH)]
        bxtf = [bpack[:, m * NB + 1 : m * NB + 2] for m in range(KH)]
        boute = [bpack[:, m * NB + 2 : m * NB + 3] for m in range(KH)]
        if bias_fix:
            cb1 = [bpack[:, m * NB + 3 : m * NB + 4] for m in range(KH)]
            cq1 = [bpack[:, m * NB + 4 : m * NB + 5] for m in range(KH)]

        for k in range(KD):
            nc.gpsimd.dma_start(wqc[k][:, :], wqc_d[ts(k, 128), :])
        nc.gpsimd.dma_start(bpack[:, :], bp_d[:, :])
        for k in range(KD):
            nc.gpsimd.dma_start(wcb[k][:, :], wcb_d[ts(k, 128), :])
        for m in range(2):
            nc.gpsimd.dma_start(kt[m][:, :], kt_d[ts(m, 128), :])
        for m in range(KD):
            nc.gpsimd.dma_start(vva[m][:, :], vva_d[ts(m, 128), :])
        nc.gpsimd.dma_start(onesb[:, :], ones_d[:, :])
        for m in range(2, KH):
            nc.gpsimd.dma_start(kt[m][:, :], kt_d[ts(m, 128), :])
        for k in range(KH):
            nc.gpsimd.dma_start(wout[k][:, :], wot_d[ts(k, 128), :])

        # absorb the one-time ACT function-table load off the critical path
        warm = wtile([1, 2], F32, "warm")
        nc.vector.memset(warm[:, 0:1], 0.0)
        nc.scalar.activation(warm[:, 1:2], warm[:, 0:1], AF.Exp, scale=1.0)

        # ---------------- per-graph emission ----------------
        def emit_xt_dma(g):
            xts = []
            for k in range(KD):
                t = dp.tile([128, F], WT, name="xtin", tag="xtin", bufs=6)
                nc.sync.dma_start(t[:, :], xt_d[g, ts(k, 128), :])
                xts.append(t)
            return xts

        def emit_front_q(g, xts, m):
            ps = ptile([128, F], "op", 4)
            for k in range(KD):
                nc.tensor.matmul(
                    ps[:, :], wqc[k][:, ts(m, 128)], xts[k][:, :],
                    start=(k == 0), stop=(k == KD - 1),
                )
            qt = dp.tile([128, F], WT, name="qt", tag="qt", bufs=12)
            if m % 2 == 0:
                nc.scalar.activation(
                    qt[:, :], ps[:, :], AF.Identity, bias=bqc[m][:, :], scale=1.0
                )
            else:
                nc.vector.tensor_scalar_add(qt[:, :], ps[:, :], bqc[m][:, :])
            if bias_fix:
                nc.vector.scalar_tensor_tensor(
                    qt[:, 1:2], cq1[m][:, :], 1.0, qt[:, 1:2], ALU.mult, ALU.add
                )
            return qt

        def emit_front_h(g, xts, m):
            ps = ptile([128, F], "op", 4)
            for k in range(KD):
                nc.tensor.matmul(
                    ps[:, :], wcb[k][:, ts(m, 128)], xts[k][:, :],
                    start=(k == 0), stop=(k == KD - 1),
                )
            xo = dp.tile([128, F], BF16, name="xtime", tag="xtime", bufs=6)
            nc.scalar.activation(
                xo[:, :], ps[:, :], AF.Identity, bias=bxtf[m][:, :], scale=1.0
            )
            if bias_fix:
                nc.vector.scalar_tensor_tensor(
                    xo[:, 1:2], cb1[m][:, :], 1.0, xo[:, 1:2], ALU.mult, ALU.add
                )
            nc.sync.dma_start(oxt_d[g, ts(m, 128), :], xo[:, :])

        def emit_scores(j, qts):
            exps = []
            for hh in range(2):
                r = DH * hh
                sc = ptile([128, 2 * F], "score", 2)
                for vh in range(2):
                    nc.tensor.matmul(
                        sc[:, ts(vh, F)],
                        kt[j][r : r + DH, ts(vh, 128)],
                        qts[j][r : r + DH, :],
                        start=True, stop=True,
                    )
                ex = dp.tile([128, 2 * F], WT, name="exp", tag="exp", bufs=6)
                nc.scalar.activation(ex[:, :], sc[:, :], AF.Exp, scale=0.125)
                exps.append(ex)
            return exps

        def emit_tail_a(g, j, exps):
            ops, rcs = [], []
            for hh in range(2):
                h = 2 * j + hh
                op = ptile([128, F], "op", 4)
                for vh in range(2):
                    nc.tensor.matmul(
                        op[0 : DH + 1, :],
                        vva[vh][:, ds((DH + 1) * h, DH + 1)],
                        exps[hh][:, ts(vh, F)],
                        start=(vh == 0), stop=(vh == 1),
                    )
                rc = dp.tile([1, F], WT, name="rc", tag=f"rc{hh}", bufs=4)
                with tc.high_priority(offset=15):
                    with nc.allow_low_precision(reason="softmax recip to bf16"):
                        nc.vector.reciprocal(rc[:, :], op[DH : DH + 1, :])
                ops.append(op)
                rcs.append(rc)
            rcb = ptile([128, F], "op", 4)
            return ops, rcs, rcb

        def emit_tail_b(j, ops_rc):
            ops, rcs, rcb = ops_rc
            ot = dp.tile([128, F], WT, name="ot", tag="ot", bufs=18)
            for hh in range(2):
                nc.tensor.matmul(
                    rcb[ts(hh, DH), :], onesb[:, :], rcs[hh][:, :],
                    start=True, stop=True, tile_position=(0, DH * hh),
                )
            # hw allows only one PSUM operand per DVE op: stage rcb in SBUF
            rcs_b = dp.tile([128, F], WT, name="rcsb", tag="rcsb", bufs=8)
            with tc.high_priority(offset=15):
                if j % 2 == 0:
                    nc.scalar.activation(rcs_b[:, :], rcb[:, :], AF.Identity)
                else:
                    nc.vector.tensor_copy(rcs_b[:, :], rcb[:, :])
            for hh in range(2):
                nc.vector.tensor_tensor(
                    ot[DH * hh : DH * hh + DH, :], ops[hh][0:DH, :],
                    rcs_b[DH * hh : DH * hh + DH, :], ALU.mult,
                )
            return ot

        def emit_xout_m(g, ots, m):
            ps = ptile([128, F], "op", 4)
            for k in range(KH):
                nc.tensor.matmul(
                    ps[:, :], wout[k][:, ts(m, 128)], ots[k][:, :],
                    start=(k == 0), stop=(k == KH - 1),
                )
            xo2 = dp.tile([128, F], BF16, name="xout", tag="xout", bufs=6)
            if m % 2 == 0:
                nc.scalar.activation(
                    xo2[:, :], ps[:, :], AF.Identity, bias=boute[m][:, :],
                    scale=1.0,
                )
            else:
                nc.vector.tensor_scalar_add(xo2[:, :], ps[:, :], boute[m][:, :])
            nc.sync.dma_start(oxo_d[g, ts(m, 128), :], xo2[:, :])

        # ---------------- interleaved software-pipelined graph loop --------
        # Per graph g, the attention pairs are interleaved with "filler"
        # chunks: the front matmuls of graph g+1 and the out-projection of
        # graph g-1.  PE then always has independent work while DVE/ACT
        # digest the recip/exp/normalize chain of the current pair.
        qts_of, ots_of = {}, {}

        def make_front_chunks(g, xts):
            qts = []
            qts_of[g] = qts

            def first():
                qts.append(emit_front_q(g, xts, 0))

            chunks = [first]
            for m in range(1, KH):
                chunks.append(lambda m=m: qts.append(emit_front_q(g, xts, m)))
            for m in range(KH):
                chunks.append(lambda m=m: emit_front_h(g, xts, m))
            return chunks

        def make_xout_chunks(g):
            return [lambda m=m: emit_xout_m(g, ots_of[g], m) for m in range(KH)]

        xts0 = emit_xt_dma(0)
        for c in make_front_chunks(0, xts0):
            c()
        carry = []
        for g in range(G):
            qts = qts_of[g]
            ots = [None] * NPAIR
            ots_of[g] = ots
            filler = list(carry)
            carry = []
            if g + 1 < G:
                xts_n = emit_xt_dma(g + 1)
                cn = make_front_chunks(g + 1, xts_n)
                if g + 1 == G - 1:
                    # the last graph's x_time chunks carry into its own
                    # attention as filler (no ordering constraint there),
                    # feeding the otherwise-starved final pairs
                    filler += cn[:KH]
                    carry = cn[KH:]
                else:
                    filler += cn
            if g >= 1:
                filler += make_xout_chunks(g - 1)
            fi = 0
            slot = 0
            NSLOT = 3 * (NPAIR - 1) + 3

            def fill(n):
                nonlocal fi, slot
                slot += n
                if g == G - 1:
                    # pipeline drain: ration the scarce filler evenly so the
                    # final pairs keep the PE fed
                    want = (slot * len(filler) + NSLOT + 1) // (NSLOT + 2)
                else:
                    want = fi + n
                while fi < min(want, len(filler)):
                    filler[fi]()
                    fi += 1

            exps = [None] * NPAIR
            tails = [None] * NPAIR
            exps[0] = emit_scores(0, qts)
            fill(2)
            for j in range(1, NPAIR):
                tails[j - 1] = emit_tail_a(g, j - 1, exps[j - 1])
                exps[j] = emit_scores(j, qts)
                fill(1)
                ots[j - 1] = emit_tail_b(j - 1, tails[j - 1])
                fill(2)
            tails[NPAIR - 1] = emit_tail_a(g, NPAIR - 1, exps[NPAIR - 1])
            fill(1)
            ots[NPAIR - 1] = emit_tail_b(NPAIR - 1, tails[NPAIR - 1])
            while fi < len(filler):
                filler[fi]()
                fi += 1
        for c in make_xout_chunks(G - 1):
            c()

    return nc


def _split_multi_waits(json_bytes):
    """Hoist extra sync waits into standalone EventSemaphore instructions.

    This walrus build encodes at most one (wait, update) pair per TPB
    instruction; Tile emits multi-entry on_wait lists, which fail codegen
    with "Too many sync wait commands". Keeping one wait inline and issuing
    the rest as same-engine EventSemaphore instructions immediately before
    is semantically identical (per-engine program order is preserved).
    """
    import orjson

    d = orjson.loads(json_bytes)
    n = 0
    for fn in d["functions"]:
        for blk in fn["blocks"]:
            out = []
            for inst in blk["instructions"]:
                sync = inst.get("sync_info")
                waits = (sync or {}).get("on_wait") or []
                if len(waits) > 1:
                    for w in waits[:-1]:
                        n += 1
                        out.append({
                            "debug": inst.get("debug", 0),
                            "engine": inst["engine"],
                            "ins": [],
                            "name": f"eswait_{n}_{inst['name']}",
                            "opcode": "EventSemaphore",
                            "outs": [],
                            "sync_info": {"on_update": [], "on_wait": [w]},
                        })
                    sync["on_wait"] = [waits[-1]]
                out.append(inst)
            blk["instructions"] = out
    return orjson.dumps(d)


_NC_CACHE = {}


def _get_nc(bias_fix=False):
    if bias_fix not in _NC_CACHE:
        nc = build_program(bias_fix=bias_fix)
        orig = nc.to_json_bytes
        nc.to_json_bytes = lambda: _split_multi_waits(orig())
        _NC_CACHE[bias_fix] = nc
    return _NC_CACHE[bias_fix]


def make_in_maps(x, word_embedding, W_lin, b_lin, W_gcn, b_gcn,
                 in_proj_w, in_proj_b, out_proj_w, out_proj_b):
    f32 = lambda a: np.ascontiguousarray(np.asarray(a), dtype=np.float32)
    wt = lambda a: np.ascontiguousarray(np.asarray(a, dtype=np.float32)).astype(WT_NP)
    x = f32(x)
    WE = f32(word_embedding)
    W_lin, W_gcn, Wout = f32(W_lin), f32(W_gcn), f32(out_proj_w)
    b_lin, b_gcn, b_out = f32(b_lin), f32(b_gcn), f32(out_proj_b)
    ipw, ipb = np.asarray(in_proj_w), np.asarray(in_proj_b)
    Wq, Wk, Wv = (f32(ipw[i * H : (i + 1) * H]) for i in range(3))
    bq, bk, bv = (f32(ipb[i * H : (i + 1) * H]) for i in range(3))

    Wcomb = W_gcn @ W_lin                       # [H, D]
    Wqcomb = Wq @ Wcomb                         # [H, D]
    ktm = Wk @ WE + bk[:, None]                 # [H, V]
    vvh = WE.T @ Wv.T                           # [V, H]
    vva = np.ones((V, NH * (DH + 1)), np.float32)
    for h in range(NH):
        vva[:, (DH + 1) * h : (DH + 1) * h + DH] = vvh[:, DH * h : DH * (h + 1)]

    gb = W_gcn @ b_lin                          # agg-uniform part of b_lin
    bxtf = gb + b_gcn
    bqc = Wq @ bxtf + bq
    boute = Wout @ bv + b_out
    cb1 = C1 * gb
    cq1 = Wq @ cb1
    bias_fix = bool(np.any(b_lin))

    xa = x.copy()
    xa[:, 1] = 0.5 * x[:, 1] + RSQRT2 * x[:, 0]
    xa[:, 2:5] = 0.5 * x[:, 2:5] + 0.5 * x[:, 1:4]
    xT = xa.reshape(NCORES, G, F, D).transpose(0, 1, 3, 2)  # [cores, G, D, F]
    NB = 5 if bias_fix else 3
    cols = [bqc, bxtf, boute] + ([cb1, cq1] if bias_fix else [])
    bpk = np.zeros((128, KH * NB), np.float32)
    for m in range(KH):
        for b, v in enumerate(cols):
            bpk[:, m * NB + b] = v[m * 128 : (m + 1) * 128]
    shared = dict(
        w_qcomb=wt(Wqcomb.T),
        w_comb=wt(Wcomb.T),
        w_out_t=wt(Wout.T),
        k_t=wt(ktm),
        vvh_a=wt(vva),
        ones_r=np.ones((1, DH), np.float32).astype(WT_NP),
        b_pack=bpk,
    )
    return bias_fix, [
        dict(shared, xt=np.ascontiguousarray(xT[c]).astype(WT_NP))
        for c in range(NCORES)
    ]


def gather_outputs(results):
    xt = np.concatenate(
        [np.asarray(r["out_xt"]).astype(np.float32).transpose(0, 2, 1)
         for r in results], axis=0
    )
    xo = np.concatenate(
        [np.asarray(r["out_xo"]).astype(np.float32).transpose(0, 2, 1)
         for r in results], axis=0
    )
    return np.ascontiguousarray(xt), np.ascontiguousarray(xo)


def kernel(**inputs):
    from concourse.bass_utils import run_bass_kernel_spmd

    bias_fix, in_maps = make_in_maps(**inputs)
    nc = _get_nc(bias_fix)
    res = run_bass_kernel_spmd(nc, in_maps, list(range(NCORES)))
    return gather_outputs(res.results)



# revision 2
# speedup vs baseline: 1.0858x; 1.0858x over previous
"""Trainium2 Bass kernel for nn_Encoder_trace (GNN message passing + cross-attention).

Data-parallel over the batch axis B=64 across 8 NeuronCores (8 graphs/core).
Device layout: channels on SBUF partitions, tokens on the free dimension
(everything computed transposed; host un-transposes on gather).

All weight-combination products AND the chain-GCN token aggregation are
precomputed on the HOST; every matmul operand is bf16.  Per graph the device
runs only the streaming work, software-pipelined so the front matmuls of
graph g+1 and the out-projection of graph g-1 interleave with the attention
pairs of graph g.

Math per graph (g), with xa = agg(x) done on host:
  x_timeT = W_comb @ xaT + bxtf            (W_comb  = W_gcn W_lin)
  qT      = W_qcomb @ xaT + bqc            (W_qcomb = Wq W_gcn W_lin)
  kT      = Wk @ WE + bk                   [host]
  vvh     = WE.T @ Wv.T, ones col interleaved per head  [host]
  per head-pair: scoresT = kT_h.T @ qT_h ; exp (ACT, scale=1/8, no max-sub)
            opa[0:65, 2F] = [vvh_h | 1].T @ exp for both heads (one 2-bank
            PSUM tile; column 64 = softmax sums, free via the ones col)
            rc = recip(opa[64, 0:2F])   (one DVE op for the pair, fp32)
            rcb[0:64]/[64:128] <- DMA partition-broadcast of rc halves
            (SBUF->SBUF DMA with 0-stride mid dim; replaces PE broadcast
            matmul + PSUM->SBUF staging copy of the old scheme)
            ot = opa[0:64] * rcb         (2 DVE mults, PSUM x SBUF)
  x_outT  = W_out @ oT + boute             (boute = W_out bv + b_out)
"""

import numpy as np
from contextlib import ExitStack

import concourse.bass as bass
import concourse.mybir as mybir
import concourse.tile as tile
from concourse.bass import ts, ds

# problem dims (hardcoded per spec)
B, F, D, H, NH, DH, V = 64, 512, 256, 768, 12, 64, 256
NCORES = 8
G = B // NCORES       # graphs per core
KH = H // 128         # 6  (H in 128-partition tiles)
KD = D // 128         # 2  (D in 128-partition tiles)
NPAIR = NH // 2       # 6  head pairs

F32 = mybir.dt.float32
BF16 = mybir.dt.bfloat16
AF = mybir.ActivationFunctionType
ALU = mybir.AluOpType

WT = BF16
WT_NP = mybir.dt.np(WT)

RSQRT2 = float(2.0 ** -0.5)
C1 = RSQRT2 - 0.5  # chain-GCN col-1 colsum deviation (bias correction coeff)


def build_program(bias_fix=False):
    nc = bass.Bass()

    xt_d = nc.declare_dram_parameter("xt", [G, D, F], WT, isOutput=False)
    wqc_d = nc.declare_dram_parameter("w_qcomb", [D, H], WT, isOutput=False)
    wcb_d = nc.declare_dram_parameter("w_comb", [D, H], WT, isOutput=False)
    wot_d = nc.declare_dram_parameter("w_out_t", [H, H], WT, isOutput=False)
    kt_d = nc.declare_dram_parameter("k_t", [H, V], WT, isOutput=False)
    vva_d = nc.declare_dram_parameter("vvh_a", [D, NH * (DH + 1)], WT, isOutput=False)
    NB = 5 if bias_fix else 3
    bp_d = nc.declare_dram_parameter("b_pack", [128, KH * NB], F32, isOutput=False)
    oxt_d = nc.declare_dram_parameter("out_xt", [G, H, F], BF16, isOutput=True)
    oxo_d = nc.declare_dram_parameter("out_xo", [G, H, F], BF16, isOutput=True)

    with ExitStack() as ctx:
        tc = ctx.enter_context(tile.TileContext(nc))
        wp = ctx.enter_context(tc.tile_pool(name="wp", bufs=1))
        pp = ctx.enter_context(tc.tile_pool(name="pp", bufs=1, space="PSUM"))
        dp = ctx.enter_context(tc.tile_pool(name="dp", bufs=1))

        def ptile(shape, tag, bufs):
            return pp.tile(shape, F32, name=tag, tag=tag, bufs=bufs)

        def wtile(shape, dt, tag):
            return wp.tile(shape, dt, name=tag, tag=tag)

        # ---------------- persistent weights (DMA in consumer order) -------
        wqc = [wtile([128, H], WT, f"wqc{k}") for k in range(KD)]
        wcb = [wtile([128, H], WT, f"wcb{k}") for k in range(KD)]
        kt = [wtile([128, V], WT, f"kt{m}") for m in range(KH)]
        vva = [wtile([128, NH * (DH + 1)], WT, f"vva{m}") for m in range(KD)]
        wout = [wtile([128, H], WT, f"wout{k}") for k in range(KH)]
        bpack = wtile([128, KH * NB], F32, "bpack")
        bqc = [bpack[:, m * NB : m * NB + 1] for m in range(KH)]
        bxtf = [bpack[:, m * NB + 1 : m * NB + 2] for m in range(KH)]
        boute = [bpack[:, m * NB + 2 : m * NB + 3] for m in range(KH)]
        if bias_fix:
            cb1 = [bpack[:, m * NB + 3 : m * NB + 4] for m in range(KH)]
            cq1 = [bpack[:, m * NB + 4 : m * NB + 5] for m in range(KH)]

        for k in range(KD):
            nc.gpsimd.dma_start(wqc[k][:, :], wqc_d[ts(k, 128), :])
        nc.gpsimd.dma_start(bpack[:, :], bp_d[:, :])
        for k in range(KD):
            nc.gpsimd.dma_start(wcb[k][:, :], wcb_d[ts(k, 128), :])
        for m in range(2):
            nc.gpsimd.dma_start(kt[m][:, :], kt_d[ts(m, 128), :])
        for m in range(KD):
            nc.gpsimd.dma_start(vva[m][:, :], vva_d[ts(m, 128), :])
        for m in range(2, KH):
            nc.gpsimd.dma_start(kt[m][:, :], kt_d[ts(m, 128), :])
        for k in range(KH):
            nc.gpsimd.dma_start(wout[k][:, :], wot_d[ts(k, 128), :])

        # absorb the one-time ACT function-table load off the critical path
        warm = wtile([1, 2], F32, "warm")
        nc.vector.memset(warm[:, 0:1], 0.0)
        nc.scalar.activation(warm[:, 1:2], warm[:, 0:1], AF.Exp, scale=1.0)

        # ---------------- per-graph emission ----------------
        def emit_xt_dma(g):
            xts = []
            for k in range(KD):
                t = dp.tile([128, F], WT, name="xtin", tag="xtin", bufs=6)
                nc.sync.dma_start(t[:, :], xt_d[g, ts(k, 128), :])
                xts.append(t)
            return xts

        def emit_front_q(g, xts, m):
            ps = ptile([128, F], "op", 2)
            for k in range(KD):
                nc.tensor.matmul(
                    ps[:, :], wqc[k][:, ts(m, 128)], xts[k][:, :],
                    start=(k == 0), stop=(k == KD - 1),
                )
            qt = dp.tile([128, F], WT, name="qt", tag="qt", bufs=12)
            if m % 2 == 0:
                nc.scalar.activation(
                    qt[:, :], ps[:, :], AF.Identity, bias=bqc[m][:, :], scale=1.0
                )
            else:
                nc.vector.tensor_scalar_add(qt[:, :], ps[:, :], bqc[m][:, :])
            if bias_fix:
                nc.vector.scalar_tensor_tensor(
                    qt[:, 1:2], cq1[m][:, :], 1.0, qt[:, 1:2], ALU.mult, ALU.add
                )
            return qt

        def emit_front_h(g, xts, m):
            ps = ptile([128, F], "op", 2)
            for k in range(KD):
                nc.tensor.matmul(
                    ps[:, :], wcb[k][:, ts(m, 128)], xts[k][:, :],
                    start=(k == 0), stop=(k == KD - 1),
                )
            xo = dp.tile([128, F], BF16, name="xtime", tag="xtime", bufs=6)
            nc.scalar.activation(
                xo[:, :], ps[:, :], AF.Identity, bias=bxtf[m][:, :], scale=1.0
            )
            if bias_fix:
                nc.vector.scalar_tensor_tensor(
                    xo[:, 1:2], cb1[m][:, :], 1.0, xo[:, 1:2], ALU.mult, ALU.add
                )
            nc.sync.dma_start(oxt_d[g, ts(m, 128), :], xo[:, :])

        def emit_scores(j, qts):
            exps = []
            for hh in range(2):
                r = DH * hh
                sc = ptile([128, 2 * F], "score", 1)
                for vh in range(2):
                    nc.tensor.matmul(
                        sc[:, ts(vh, F)],
                        kt[j][r : r + DH, ts(vh, 128)],
                        qts[j][r : r + DH, :],
                        start=True, stop=True,
                    )
                ex = dp.tile([128, 2 * F], WT, name="exp", tag="exp", bufs=6)
                nc.scalar.activation(ex[:, :], sc[:, :], AF.Exp, scale=0.125)
                exps.append(ex)
            return exps

        def emit_tail_a(g, j, exps):
            # both heads of the pair into one 2-bank PSUM tile:
            # head hh data+sums at columns [hh*F, hh*F+F)
            opa = ptile([128, 2 * F], "opa", 2)
            for hh in range(2):
                h = 2 * j + hh
                for vh in range(2):
                    nc.tensor.matmul(
                        opa[0 : DH + 1, ts(hh, F)],
                        vva[vh][:, ds((DH + 1) * h, DH + 1)],
                        exps[hh][:, ts(vh, F)],
                        start=(vh == 0), stop=(vh == 1),
                    )
            # one fp32 reciprocal for the pair's two sum rows [1, 2F]
            rc = dp.tile([1, 2 * F], F32, name="rc", tag="rc", bufs=3)
            with tc.high_priority(offset=15):
                nc.vector.reciprocal(rc[:, :], opa[DH : DH + 1, :])
            # partition-broadcast each half across 64 partitions via
            # SBUF->SBUF DMA (0-stride middle dim) on otherwise-idle queues
            rcb = dp.tile([128, F], F32, name="rcb", tag="rcb", bufs=3)
            for hh in range(2):
                src = rc[0:1, ts(hh, F)].unsqueeze(1).broadcast_to((1, DH, F))
                nc.gpsimd.dma_start(rcb[ts(hh, DH), :], src)
            return opa, rcb

        def emit_tail_b(j, tail):
            opa, rcb = tail
            ot = dp.tile([128, F], WT, name="ot", tag="ot", bufs=18)
            for hh in range(2):
                nc.vector.tensor_tensor(
                    ot[ts(hh, DH), :], opa[0:DH, ts(hh, F)],
                    rcb[ts(hh, DH), :], ALU.mult,
                )
            return ot

        def emit_xout_m(g, ots, m):
            ps = ptile([128, F], "op", 2)
            for k in range(KH):
                nc.tensor.matmul(
                    ps[:, :], wout[k][:, ts(m, 128)], ots[k][:, :],
                    start=(k == 0), stop=(k == KH - 1),
                )
            xo2 = dp.tile([128, F], BF16, name="xout", tag="xout", bufs=6)
            if m % 2 == 0:
                nc.scalar.activation(
                    xo2[:, :], ps[:, :], AF.Identity, bias=boute[m][:, :],
                    scale=1.0,
                )
            else:
                nc.vector.tensor_scalar_add(xo2[:, :], ps[:, :], boute[m][:, :])
            nc.sync.dma_start(oxo_d[g, ts(m, 128), :], xo2[:, :])

        # ---------------- interleaved software-pipelined graph loop --------
        # Per graph g, the attention pairs are interleaved with "filler"
        # chunks: the front matmuls of graph g+1 and the out-projection of
        # graph g-1.  The per-pair normalize chain (recip -> broadcast DMA
        # -> mult) has ~2.5us of DMA latency, so the mults for pair j are
        # emitted one pair late (tail_b(j-1) alongside tail_a(j)).
        qts_of, ots_of = {}, {}

        def make_front_chunks(g, xts):
            qts = []
            qts_of[g] = qts

            def first():
                qts.append(emit_front_q(g, xts, 0))

            chunks = [first]
            for m in range(1, KH):
                chunks.append(lambda m=m: qts.append(emit_front_q(g, xts, m)))
            for m in range(KH):
                chunks.append(lambda m=m: emit_front_h(g, xts, m))
            return chunks

        def make_xout_chunks(g):
            return [lambda m=m: emit_xout_m(g, ots_of[g], m) for m in range(KH)]

        xts0 = emit_xt_dma(0)
        for c in make_front_chunks(0, xts0):
            c()
        carry = []
        for g in range(G):
            qts = qts_of[g]
            ots = [None] * NPAIR
            ots_of[g] = ots
            filler = list(carry)
            carry = []
            if g + 1 < G:
                xts_n = emit_xt_dma(g + 1)
                cn = make_front_chunks(g + 1, xts_n)
                if g + 1 == G - 1:
                    # the last graph's x_time chunks carry into its own
                    # attention as filler (no ordering constraint there),
                    # feeding the otherwise-starved final pairs
                    filler += cn[:KH]
                    carry = cn[KH:]
                else:
                    filler += cn
            if g >= 1:
                filler += make_xout_chunks(g - 1)
            fi = 0
            slot = 0
            NSLOT = 3 * (NPAIR - 1) + 3

            def fill(n):
                nonlocal fi, slot
                slot += n
                if g == G - 1:
                    # pipeline drain: ration the scarce filler evenly so the
                    # final pairs keep the PE fed
                    want = (slot * len(filler) + NSLOT + 1) // (NSLOT + 2)
                else:
                    want = fi + n
                while fi < min(want, len(filler)):
                    filler[fi]()
                    fi += 1

            exps = [None] * NPAIR
            tails = [None] * NPAIR
            exps[0] = emit_scores(0, qts)
            fill(2)
            for j in range(1, NPAIR):
                tails[j - 1] = emit_tail_a(g, j - 1, exps[j - 1])
                exps[j] = emit_scores(j, qts)
                fill(1)
                if j >= 2:
                    ots[j - 2] = emit_tail_b(j - 2, tails[j - 2])
                fill(2)
            tails[NPAIR - 1] = emit_tail_a(g, NPAIR - 1, exps[NPAIR - 1])
            fill(1)
            ots[NPAIR - 2] = emit_tail_b(NPAIR - 2, tails[NPAIR - 2])
            fill(1)
            ots[NPAIR - 1] = emit_tail_b(NPAIR - 1, tails[NPAIR - 1])
            while fi < len(filler):
                filler[fi]()
                fi += 1
        for c in make_xout_chunks(G - 1):
            c()

    return nc


def _split_multi_waits(json_bytes):
    """Hoist extra sync waits into standalone EventSemaphore instructions.

    This walrus build encodes at most one (wait, update) pair per TPB
    instruction; Tile emits multi-entry on_wait lists, which fail codegen
    with "Too many sync wait commands". Keeping one wait inline and issuing
    the rest as same-engine EventSemaphore instructions immediately before
    is semantically identical (per-engine program order is preserved).
    """
    import orjson

    d = orjson.loads(json_bytes)
    n = 0
    for fn in d["functions"]:
        for blk in fn["blocks"]:
            out = []
            for inst in blk["instructions"]:
                sync = inst.get("sync_info")
                waits = (sync or {}).get("on_wait") or []
                if len(waits) > 1:
                    for w in waits[:-1]:
                        n += 1
                        out.append({
                            "debug": inst.get("debug", 0),
                            "engine": inst["engine"],
                            "ins": [],
                            "name": f"eswait_{n}_{inst['name']}",
                            "opcode": "EventSemaphore",
                            "outs": [],
                            "sync_info": {"on_update": [], "on_wait": [w]},
                        })
                    sync["on_wait"] = [waits[-1]]
                out.append(inst)
            blk["instructions"] = out
    return orjson.dumps(d)


_NC_CACHE = {}


def _get_nc(bias_fix=False):
    if bias_fix not in _NC_CACHE:
        nc = build_program(bias_fix=bias_fix)
        orig = nc.to_json_bytes
        nc.to_json_bytes = lambda: _split_multi_waits(orig())
        _NC_CACHE[bias_fix] = nc
    return _NC_CACHE[bias_fix]


def make_in_maps(x, word_embedding, W_lin, b_lin, W_gcn, b_gcn,
                 in_proj_w, in_proj_b, out_proj_w, out_proj_b):
    f32 = lambda a: np.ascontiguousarray(np.asarray(a), dtype=np.float32)
    wt = lambda a: np.ascontiguousarray(np.asarray(a, dtype=np.float32)).astype(WT_NP)
    x = f32(x)
    WE = f32(word_embedding)
    W_lin, W_gcn, Wout = f32(W_lin), f32(W_gcn), f32(out_proj_w)
    b_lin, b_gcn, b_out = f32(b_lin), f32(b_gcn), f32(out_proj_b)
    ipw, ipb = np.asarray(in_proj_w), np.asarray(in_proj_b)
    Wq, Wk, Wv = (f32(ipw[i * H : (i + 1) * H]) for i in range(3))
    bq, bk, bv = (f32(ipb[i * H : (i + 1) * H]) for i in range(3))

    Wcomb = W_gcn @ W_lin                       # [H, D]
    Wqcomb = Wq @ Wcomb                         # [H, D]
    ktm = Wk @ WE + bk[:, None]                 # [H, V]
    vvh = WE.T @ Wv.T                           # [V, H]
    vva = np.ones((V, NH * (DH + 1)), np.float32)
    for h in range(NH):
        vva[:, (DH + 1) * h : (DH + 1) * h + DH] = vvh[:, DH * h : DH * (h + 1)]

    gb = W_gcn @ b_lin                          # agg-uniform part of b_lin
    bxtf = gb + b_gcn
    bqc = Wq @ bxtf + bq
    boute = Wout @ bv + b_out
    cb1 = C1 * gb
    cq1 = Wq @ cb1
    bias_fix = bool(np.any(b_lin))

    xa = x.copy()
    xa[:, 1] = 0.5 * x[:, 1] + RSQRT2 * x[:, 0]
    xa[:, 2:5] = 0.5 * x[:, 2:5] + 0.5 * x[:, 1:4]
    xT = xa.reshape(NCORES, G, F, D).transpose(0, 1, 3, 2)  # [cores, G, D, F]
    NB = 5 if bias_fix else 3
    cols = [bqc, bxtf, boute] + ([cb1, cq1] if bias_fix else [])
    bpk = np.zeros((128, KH * NB), np.float32)
    for m in range(KH):
        for b, v in enumerate(cols):
            bpk[:, m * NB + b] = v[m * 128 : (m + 1) * 128]
    shared = dict(
        w_qcomb=wt(Wqcomb.T),
        w_comb=wt(Wcomb.T),
        w_out_t=wt(Wout.T),
        k_t=wt(ktm),
        vvh_a=wt(vva),
        b_pack=bpk,
    )
    return bias_fix, [
        dict(shared, xt=np.ascontiguousarray(xT[c]).astype(WT_NP))
        for c in range(NCORES)
    ]


def gather_outputs(results):
    xt = np.concatenate(
        [np.asarray(r["out_xt"]).astype(np.float32).transpose(0, 2, 1)
         for r in results], axis=0
    )
    xo = np.concatenate(
        [np.asarray(r["out_xo"]).astype(np.float32).transpose(0, 2, 1)
         for r in results], axis=0
    )
    return np.ascontiguousarray(xt), np.ascontiguousarray(xo)


def kernel(**inputs):
    from concourse.bass_utils import run_bass_kernel_spmd

    bias_fix, in_maps = make_in_maps(**inputs)
    nc = _get_nc(bias_fix)
    res = run_bass_kernel_spmd(nc, in_maps, list(range(NCORES)))
    return gather_outputs(res.results)
